# revision 1
# baseline (speedup 1.0000x reference)
"""GAT layer (gnn_message_passing) Trainium2 Bass kernel.

Per-core work (data-parallel over batch B=8, one graph per NeuronCore):
  h   = (x*m) @ W
  e   = leakyrelu(e_l[i] + e_r[j]),  e_l = h@a_l, e_r = h@a_r
  p   = adj*mask_j ? exp(e) : 0        (softmax numerator; exp(-1e4) == 0)
  out = LN(((p @ h) / rowsum(p) + x*m) * m) * gamma + beta

Device layout trick: the host feeds adj TRANSPOSED as bf16 {0,1} so the
[N,N] pipeline runs in [j-partition, i-free] orientation and the big
matmul p@h needs no on-chip transpose (contraction over j = partitions).
mask_j is folded additively into e_r (e_r - 1e4*(1-m_j)); mask_i is not
needed (masked rows are zeroed by the final *m).
"""

import os
import sys

import numpy as np

if "/opt/trn_rl_repo" not in sys.path:
    sys.path.insert(0, "/opt/trn_rl_repo")

B, N, D = 8, 2048, 128
NB = N // 128
ALPHA = 0.2
EPS = 1e-5
NCORES = 8

# Perf knobs: which j-blocks compute leakyrelu on DVE (3-op max trick)
# instead of ScalarE Lrelu, and which do the adj-mask multiply on GPSIMD.
DVE_LRELU_BLOCKS = frozenset({0, 3, 6, 9, 12, 15})
GPSIMD_MASK_BLOCKS = frozenset()

_PROG_CACHE = {}
RACE_DETECT = True  # sim_check disables: tail sem-decrements trip the sim's race detector
SEM_CLEAR_MODE = "skip"  # tail sem reset unnecessary (runtime resets between executions);
# "dec" variant crashes the device, "skip" verified correct across reruns
LAST_EXEC_TIME_NS = None
LAST_MEAN_EXEC_TIME_NS = None


def _patch_sem_clear():
    """This environment's walrus rejects EVENT_SEMAPHORE_RANGE_CLEAR
    ("ISA wrong length" — ISA table skew vs the repo).  Replace Tile's
    tail range-clear with per-semaphore decrements of each semaphore's
    statically-known net increment, which is equivalent for a
    deterministic program (each execution starts from the cleared
    state the previous one restored).
    """
    import bass_rust
    import concourse.bass as bass

    if getattr(bass.BassEngine, "_gat_sem_clear_patched", False):
        return

    def sem_clear(self, sem):
        if SEM_CLEAR_MODE == "skip":
            return None
        if not isinstance(sem, range):
            sem = range(sem.num, sem.num + 1)
        net = {s: 0 for s in sem}
        for b in self.bass.m.functions[0].blocks:
            for inst in b.instructions:
                si = inst.sync_info
                if si is None or not si.on_update:
                    continue
                for u in si.on_update:
                    if u.id in net:
                        if u.update_mode in ("sem-add-imm", "sem-inc"):
                            net[u.id] += u.update_value if u.update_value is not None else 1
                        elif u.update_mode in ("sem-dec",):
                            net[u.id] -= u.update_value if u.update_value is not None else 1
                        else:
                            raise AssertionError(u.update_mode)
        last = None
        for s in sem:
            if net[s]:
                h = bass_rust.SemaphoreHandle(name=f"semdec_{s}", num=s)
                last = self.sem_inc(h, -net[s])
        return last

    bass.BassEngine.sem_clear = sem_clear
    bass.BassEngine._gat_sem_clear_patched = True


def _split_waits(nc, mybir, max_waits=1):
    """This walrus build allows only one semaphore-wait slot per
    instruction ("Too many sync wait commands").  Hoist extra waits onto
    standalone EventSemaphore carrier instructions placed immediately
    before the offender on the same engine; the engine sequencer
    executes them in order, so the dependency semantics are unchanged.
    """
    for f in nc.m.functions:
        for b in f.blocks:
            il = b.instructions
            k = 0
            while k < len(il):
                i = il[k]
                si = i.sync_info
                if si is not None and si.on_wait and len(si.on_wait) > max_waits:
                    waits = list(si.on_wait)
                    extra, keep = waits[:-max_waits], waits[-max_waits:]
                    for j, w in enumerate(extra):
                        ev = mybir.InstEventSemaphore(
                            name=f"{i.name}-wsplit{j}",
                            engine=i.engine,
                            debug=i.debug,
                            sync_info=mybir.SyncInfo(on_wait=[w], on_update=[]),
                        )
                        il.insert(k + j, ev)
                    k += len(extra)
                    i.sync_info = mybir.SyncInfo(
                        on_wait=keep, on_update=list(si.on_update or []))
                k += 1
    return nc


def _knobs():
    dve = os.environ.get("GAT_DVE_LRELU")
    gp = os.environ.get("GAT_GP_MASK")
    d = frozenset(int(x) for x in dve.split(",")) if dve else DVE_LRELU_BLOCKS
    g = frozenset(int(x) for x in gp.split(",")) if gp else GPSIMD_MASK_BLOCKS
    return d, g


def _build_program(apply_affine: bool):
    import concourse.bass as bass
    import concourse.tile as tile
    from concourse import mybir
    from concourse.masks import make_identity

    _patch_sem_clear()
    dve_lrelu, gp_mask = _knobs()

    fp32 = mybir.dt.float32
    bf16 = mybir.dt.bfloat16
    A = mybir.AluOpType
    F = mybir.ActivationFunctionType

    nc = bass.Bass(use_seq_codegen=True, detect_race_conditions=RACE_DETECT)

    x_in = nc.declare_dram_parameter("x", [N, D], fp32, isOutput=False)
    adjt = nc.declare_dram_parameter("adjt", [N, N], bf16, isOutput=False)
    maskf = nc.declare_dram_parameter("maskf", [N], fp32, isOutput=False)
    w_in = nc.declare_dram_parameter("w", [D, D], bf16, isOutput=False)
    al_in = nc.declare_dram_parameter("al", [D], bf16, isOutput=False)
    ar_in = nc.declare_dram_parameter("ar", [D], bf16, isOutput=False)
    if apply_affine:
        g_in = nc.declare_dram_parameter("gamma", [D], fp32, isOutput=False)
        b_in = nc.declare_dram_parameter("beta", [D], fp32, isOutput=False)
    out_d = nc.declare_dram_parameter("out", [N, D], fp32, isOutput=True)

    el_dram = nc.dram_tensor("el_scratch", [N], bf16)

    def bcast(ap, parts=128):
        """Partition-broadcast read AP (step 0 on the partition dim)."""
        return bass.AP(tensor=ap.tensor, offset=ap.offset, ap=[[0, parts]] + list(ap.ap))

    with tile.TileContext(nc) as tc:
        with tc.tile_pool(name="persist", bufs=1) as per:
            ident_bf = per.tile([128, 128], bf16)
            make_identity(nc, ident_bf)
            ident_f32 = per.tile([128, 128], fp32)
            make_identity(nc, ident_f32)
            ones_col = per.tile([128, 1], bf16)
            nc.vector.memset(ones_col, 1.0)
            eps_col = per.tile([128, 1], fp32)
            nc.vector.memset(eps_col, EPS)

            m_col = per.tile([128, NB], fp32)
            nc.sync.dma_start(out=m_col, in_=maskf[:].rearrange("(b p) -> p b", p=128))
            w_sb = per.tile([128, D], bf16)
            nc.sync.dma_start(out=w_sb, in_=w_in[:, :])
            al_bc = per.tile([128, D], bf16)
            nc.sync.dma_start(out=al_bc, in_=bcast(al_in[:]))
            ar_bc = per.tile([128, D], bf16)
            nc.sync.dma_start(out=ar_bc, in_=bcast(ar_in[:]))
            if apply_affine:
                g_bc = per.tile([128, D], fp32)
                nc.sync.dma_start(out=g_bc, in_=bcast(g_in[:]))
                b_bc = per.tile([128, D], fp32)
                nc.sync.dma_start(out=b_bc, in_=bcast(b_in[:]))

            x_tiles = [per.tile([128, D], fp32, name=f"xt{i}", tag=f"x{i}") for i in range(NB)]
            adj_tiles = [per.tile([128, N], bf16, name=f"adjt{i}", tag=f"adj{i}") for i in range(NB)]
            xm_all = per.tile([128, NB, D], fp32)     # x*m, f32 (residual)
            xmT_all = per.tile([128, NB, D], bf16)    # (x*m)^T blocks
            h_all = per.tile([128, NB, D], bf16)      # h blocks [node, d]
            el_col = per.tile([128, NB], fp32)
            er_col = per.tile([128, NB], fp32)
            er2_col = per.tile([128, NB], fp32)
            el_bc = per.tile([128, N], bf16)          # e_l broadcast over partitions
            z_all = per.tile([128, NB, D], fp32)      # pre-LN result
            o_tiles = [per.tile([128, D], fp32, name=f"ot{i}", tag=f"o{i}") for i in range(NB)]
            mv_all = per.tile([128, NB, 2], fp32)     # bn_aggr mean/var
            oT_sb = per.tile([128, N], fp32)          # (p@h)^T copy
            rs_sb = per.tile([1, N], fp32)            # rowsums

            # ---- preprocessing: xm, xm^T, h, e_l, e_r -------------------
            with (
                tc.tile_pool(name="pp", bufs=3) as pp,
                tc.tile_pool(name="pp_ps", bufs=2, space="PSUM") as pp_ps,
            ):
                for ib in range(NB):
                    x_t = x_tiles[ib]
                    nc.sync.dma_start(out=x_t, in_=x_in[ib * 128:(ib + 1) * 128, :])
                    nc.vector.tensor_scalar(
                        out=xm_all[:, ib, :], in0=x_t,
                        scalar1=m_col[:, ib:ib + 1], scalar2=None, op0=A.mult)
                    xm_bf = pp.tile([128, D], bf16, tag="xmbf")
                    nc.vector.tensor_copy(out=xm_bf, in_=xm_all[:, ib, :])
                    xmT_ps = pp_ps.tile([128, D], bf16, tag="xmT")
                    nc.tensor.transpose(xmT_ps, xm_bf, ident_bf)
                    nc.vector.tensor_copy(out=xmT_all[:, ib, :], in_=xmT_ps)
                for ib in range(NB):
                    h_ps = pp_ps.tile([128, D], fp32, tag="h")
                    nc.tensor.matmul(h_ps, lhsT=xmT_all[:, ib, :], rhs=w_sb,
                                     start=True, stop=True)
                    nc.vector.tensor_copy(out=h_all[:, ib, :], in_=h_ps)
                    hal = pp.tile([128, D], fp32, tag="hal")
                    nc.vector.tensor_tensor(out=hal, in0=h_all[:, ib, :],
                                            in1=al_bc, op=A.mult)
                    nc.vector.tensor_reduce(out=el_col[:, ib:ib + 1], in_=hal,
                                            axis=mybir.AxisListType.X,
                                            op=A.add)
                    har = pp.tile([128, D], fp32, tag="har")
                    nc.vector.tensor_tensor(out=har, in0=h_all[:, ib, :],
                                            in1=ar_bc, op=A.mult)
                    nc.vector.tensor_reduce(out=er_col[:, ib:ib + 1], in_=har,
                                            axis=mybir.AxisListType.X,
                                            op=A.add)

                # e_r2 = e_r + 1e4*m - 1e4   (mask_j folded additively)
                tmp_col = pp.tile([128, NB], fp32, tag="tmpc")
                nc.vector.tensor_scalar(out=tmp_col, in0=m_col,
                                        scalar1=1e4, scalar2=-1e4,
                                        op0=A.mult, op1=A.add)
                nc.vector.tensor_tensor(out=er2_col, in0=er_col, in1=tmp_col,
                                        op=A.add)

                # e_l column -> row (PE transpose) -> DRAM -> broadcast tile
                el_bf_col = pp.tile([128, NB], bf16, tag="elbf")
                nc.vector.tensor_copy(out=el_bf_col, in_=el_col)
                elT_ps = pp_ps.tile([NB, 128], bf16, tag="elT")
                nc.tensor.transpose(elT_ps, el_bf_col, ident_bf)
                elT_sb = pp.tile([NB, 128], bf16, tag="elTs")
                nc.vector.tensor_copy(out=elT_sb, in_=elT_ps)
                nc.gpsimd.dma_start(out=el_dram[:].rearrange("(b q) -> b q", q=128),
                                    in_=elT_sb)
                nc.gpsimd.dma_start(out=el_bc, in_=bcast(el_dram[:]))

            # ---- main loop over j-blocks --------------------------------
            with (
                tc.tile_pool(name="mm_ps", bufs=1, space="PSUM") as mm_ps_pool,
                tc.tile_pool(name="rs_ps", bufs=1, space="PSUM") as rs_ps_pool,
                tc.tile_pool(name="blk", bufs=4) as blk,
                tc.tile_pool(name="ublk", bufs=4) as ublk,
            ):
                oT_ps = mm_ps_pool.tile([128, N], fp32)
                rs_ps = rs_ps_pool.tile([1, N], fp32)
                for jb in range(NB):
                    adj_t = adj_tiles[jb]
                    nc.sync.dma_start(out=adj_t,
                                      in_=adjt[jb * 128:(jb + 1) * 128, :])
                    er2_s = er2_col[:, jb:jb + 1]
                    u = ublk.tile([128, N], bf16, tag="u")
                    if jb in dve_lrelu:
                        t2 = ublk.tile([128, N], bf16, tag="t2")
                        nc.vector.tensor_scalar(out=t2, in0=el_bc, scalar1=er2_s,
                                                scalar2=None, op0=A.add)
                        ta = ublk.tile([128, N], bf16, tag="ta")
                        nc.vector.tensor_scalar(out=ta, in0=t2, scalar1=ALPHA,
                                                scalar2=None, op0=A.mult)
                        nc.vector.tensor_tensor(out=u, in0=t2, in1=ta, op=A.max)
                    else:
                        nc.scalar.activation(out=u, in_=el_bc, func=F.Lrelu,
                                             bias=er2_s, scale=1.0, alpha=ALPHA)
                    pexp = ublk.tile([128, N], bf16, tag="pexp")
                    nc.scalar.activation(out=pexp, in_=u, func=F.Exp)
                    pm = blk.tile([128, N], bf16, tag="pm")
                    eng = nc.gpsimd if jb in gp_mask else nc.vector
                    eng.tensor_tensor(out=pm, in0=pexp, in1=adj_t, op=A.mult)

                    st, sp = (jb == 0), (jb == NB - 1)
                    for s in range(4):
                        nc.tensor.matmul(oT_ps[:, s * 512:(s + 1) * 512],
                                         lhsT=h_all[:, jb, :],
                                         rhs=pm[:, s * 512:(s + 1) * 512],
                                         start=st, stop=sp)
                    for s in range(4):
                        nc.tensor.matmul(rs_ps[:, s * 512:(s + 1) * 512],
                                         lhsT=ones_col,
                                         rhs=pm[:, s * 512:(s + 1) * 512],
                                         start=st, stop=sp)

                nc.vector.tensor_copy(out=rs_sb, in_=rs_ps)
                nc.scalar.copy(out=oT_sb, in_=oT_ps)

            # ---- epilogue: normalize, residual, layernorm ---------------
            with (
                tc.tile_pool(name="ep", bufs=4) as ep,
                tc.tile_pool(name="ep_ps", bufs=2, space="PSUM") as ep_ps,
            ):
                # rowsum row [1,N] -> col [128,NB] via bounce + PE transpose
                rsT = ep.tile([NB, 128], fp32, tag="rsT")
                nc.gpsimd.dma_start(out=rsT,
                                    in_=rs_sb.rearrange("o (b q) -> o b q", q=128))
                rsc_ps = ep_ps.tile([128, NB], fp32, tag="rsc")
                nc.tensor.transpose(rsc_ps, rsT, ident_f32[:NB, :NB])
                r_col = ep.tile([128, NB], fp32, tag="rcol")
                nc.vector.reciprocal(out=r_col, in_=rsc_ps)
                rm_col = ep.tile([128, NB], fp32, tag="rmcol")
                nc.vector.tensor_tensor(out=rm_col, in0=r_col, in1=m_col,
                                        op=A.mult)

                for ib in range(NB):
                    tr_ps = ep_ps.tile([128, 128], fp32, tag="tr")
                    nc.tensor.transpose(tr_ps, oT_sb[:, ib * 128:(ib + 1) * 128],
                                        ident_f32)
                    z1 = ep.tile([128, 128], fp32, tag="z1")
                    nc.vector.tensor_scalar(out=z1, in0=tr_ps,
                                            scalar1=rm_col[:, ib:ib + 1],
                                            scalar2=None, op0=A.mult)
                    nc.vector.tensor_tensor(out=z_all[:, ib, :], in0=z1,
                                            in1=xm_all[:, ib, :], op=A.add)
                    st6 = ep.tile([128, 6], fp32, tag="st6")
                    nc.vector.bn_stats(out=st6, in_=z_all[:, ib, :])
                    nc.vector.bn_aggr(out=mv_all[:, ib, :], in_=st6)

                # rstd = exp(-0.5*ln(var+eps)) : stays in the exp/ln table set
                var_v = mv_all[:, :, 1:2].rearrange("p b o -> p (b o)")
                lnv = ep.tile([128, NB], fp32, tag="lnv")
                nc.scalar.activation(out=lnv, in_=var_v, func=F.Ln,
                                     bias=eps_col, scale=1.0)
                rstd = ep.tile([128, NB], fp32, tag="rstd")
                nc.scalar.activation(out=rstd, in_=lnv, func=F.Exp, scale=-0.5)

                for ib in range(NB):
                    o_t = o_tiles[ib]
                    nc.vector.tensor_scalar(
                        out=o_t, in0=z_all[:, ib, :],
                        scalar1=mv_all[:, ib, 0:1].rearrange("p o -> p o"),
                        scalar2=rstd[:, ib:ib + 1],
                        op0=A.subtract, op1=A.mult)
                    if apply_affine:
                        nc.vector.tensor_tensor(out=o_t, in0=o_t, in1=g_bc,
                                                op=A.mult)
                        nc.vector.tensor_tensor(out=o_t, in0=o_t, in1=b_bc,
                                                op=A.add)
                    nc.gpsimd.dma_start(out=out_d[ib * 128:(ib + 1) * 128, :],
                                        in_=o_t)
    return _split_waits(nc, mybir)


def _get_program(apply_affine: bool):
    key = (apply_affine, _knobs())
    if key not in _PROG_CACHE:
        _PROG_CACHE[key] = _build_program(apply_affine)
    return _PROG_CACHE[key]


def _prep_inputs(x, adj_bool, node_mask, W, a_l, a_r, gamma, beta, apply_affine):
    import ml_dtypes

    bf16 = ml_dtypes.bfloat16
    x = np.asarray(x, dtype=np.float32)
    adj_bool = np.asarray(adj_bool)
    node_mask = np.asarray(node_mask)
    w_bf = np.ascontiguousarray(np.asarray(W, dtype=np.float32).astype(bf16))
    al_bf = np.ascontiguousarray(np.asarray(a_l, dtype=np.float32).astype(bf16))
    ar_bf = np.ascontiguousarray(np.asarray(a_r, dtype=np.float32).astype(bf16))
    in_maps = []
    for b in range(NCORES):
        adjt = np.ascontiguousarray(adj_bool[b].T.astype(bf16))
        m = {
            "x": np.ascontiguousarray(x[b]),
            "adjt": adjt,
            "maskf": np.ascontiguousarray(node_mask[b].astype(np.float32)),
            "w": w_bf,
            "al": al_bf,
            "ar": ar_bf,
        }
        if apply_affine:
            m["gamma"] = np.ascontiguousarray(np.asarray(gamma, np.float32))
            m["beta"] = np.ascontiguousarray(np.asarray(beta, np.float32))
        in_maps.append(m)
    return in_maps


def kernel(x, adj_bool, node_mask, W, a_l, a_r, gamma, beta):
    global LAST_EXEC_TIME_NS, LAST_MEAN_EXEC_TIME_NS
    from concourse.bass_utils import run_bass_kernel_spmd

    gamma_np = np.asarray(gamma, dtype=np.float32)
    beta_np = np.asarray(beta, dtype=np.float32)
    apply_affine = not (np.all(gamma_np == 1.0) and np.all(beta_np == 0.0))

    nc = _get_program(apply_affine)
    in_maps = _prep_inputs(x, adj_bool, node_mask, W, a_l, a_r,
                           gamma_np, beta_np, apply_affine)
    trace = bool(int(os.environ.get("GAT_TRACE", "0")))
    res = run_bass_kernel_spmd(nc, in_maps, list(range(NCORES)), trace=trace)
    LAST_EXEC_TIME_NS = res.exec_time_ns
    LAST_MEAN_EXEC_TIME_NS = res.mean_exec_time_ns
    out = np.stack([np.asarray(r["out"], dtype=np.float32) for r in res.results])
    return out



# revision 12
# speedup vs baseline: 2.1392x; 2.1392x over previous
"""GAT layer (gnn_message_passing) Trainium2 Bass kernel, v2: node-compacted.

Per-core work (data-parallel over batch B=8, one graph per NeuronCore).
Host-side LAYOUT transform: node_mask kills ~50% of nodes; masked rows
and columns of the attention matrix contribute nothing (their pm entries
are zero / their outputs are overwritten by the final mask), so the host
selects the kept node subset and ships compacted tensors:
  xk   [J, D]  kept-node features (zero-padded to J = JB*128)
  adjm [J, J]  additive mask, adjm[j, i] = 0 if edge(keep_i <- keep_j)
               else -1e4, bf16 (j = partition dim, i = free dim)
All model math runs on device, on the compacted graph:
  h    = xk @ W
  e    = lrelu(el_i + er_j + adjm_ji)   (additive mask: lrelu(-1e4+x)
                                         stays hugely negative -> exp=0)
  pm   = exp(e)                         (softmax numerator, pre-masked)
  oT   = h^T @ pm ; rs = 1^T @ pm       (PE, accumulate over j blocks)
  out  = LN(oT^T / rs + xk)
Host scatters kept rows back into the full [N, D] output (masked rows
are exactly beta = LN affine bias, zeros here).

Engine balance: lrelu runs on ScalarE (Prelu, which shares an ACT table
with Exp -> no table reloads) for ACT_LRELU_BLOCKS, on DVE (3-op max
trick) otherwise; small copies are spread over ACT/DVE/Pool.
"""

import os
import sys

import numpy as np

if "/opt/trn_rl_repo" not in sys.path:
    sys.path.insert(0, "/opt/trn_rl_repo")

B, N, D = 8, 2048, 128
ALPHA = 0.2
EPS = 1e-5
NEG = -10000.0
NCORES = 8

_PROG_CACHE = {}
RACE_DETECT = True
SEM_CLEAR_MODE = "skip"  # runtime resets sems between executions (verified)
LAST_EXEC_TIME_NS = None
LAST_MEAN_EXEC_TIME_NS = None


def _knob(name, default):
    v = os.environ.get(name)
    if v is None or v == "":
        return frozenset(default)
    if v == "-":
        return frozenset()
    return frozenset(int(x) for x in v.split(","))


def _patch_sem_clear():
    """This environment's walrus rejects EVENT_SEMAPHORE_RANGE_CLEAR
    ("ISA wrong length").  Tail sem reset is unnecessary here (runtime
    restores sems between executions), so skip it."""
    import bass_rust
    import concourse.bass as bass

    if getattr(bass.BassEngine, "_gat_sem_clear_patched", False):
        return

    def sem_clear(self, sem):
        if SEM_CLEAR_MODE == "skip":
            return None
        if not isinstance(sem, range):
            sem = range(sem.num, sem.num + 1)
        net = {s: 0 for s in sem}
        for b in self.bass.m.functions[0].blocks:
            for inst in b.instructions:
                si = inst.sync_info
                if si is None or not si.on_update:
                    continue
                for u in si.on_update:
                    if u.id in net:
                        if u.update_mode in ("sem-add-imm", "sem-inc"):
                            net[u.id] += u.update_value if u.update_value is not None else 1
                        elif u.update_mode in ("sem-dec",):
                            net[u.id] -= u.update_value if u.update_value is not None else 1
                        else:
                            raise AssertionError(u.update_mode)
        last = None
        for s in sem:
            if net[s]:
                h = bass_rust.SemaphoreHandle(name=f"semdec_{s}", num=s)
                last = self.sem_inc(h, -net[s])
        return last

    bass.BassEngine.sem_clear = sem_clear
    bass.BassEngine._gat_sem_clear_patched = True


def _split_waits(nc, mybir, max_waits=1):
    """This walrus build allows only one semaphore-wait slot per
    instruction; hoist extra waits onto standalone EventSemaphore
    carriers immediately before the offender on the same engine."""
    for f in nc.m.functions:
        for b in f.blocks:
            il = b.instructions
            k = 0
            while k < len(il):
                i = il[k]
                si = i.sync_info
                if si is not None and si.on_wait and len(si.on_wait) > max_waits:
                    waits = list(si.on_wait)
                    extra, keep = waits[:-max_waits], waits[-max_waits:]
                    for j, w in enumerate(extra):
                        ev = mybir.InstEventSemaphore(
                            name=f"{i.name}-wsplit{j}",
                            engine=i.engine,
                            debug=i.debug,
                            sync_info=mybir.SyncInfo(on_wait=[w], on_update=[]),
                        )
                        il.insert(k + j, ev)
                    k += len(extra)
                    i.sync_info = mybir.SyncInfo(
                        on_wait=keep, on_update=list(si.on_update or []))
                k += 1
    return nc


def _build_program(jb_count: int, apply_affine: bool):
    import concourse.bass as bass
    import concourse.tile as tile
    from concourse import mybir
    from concourse.masks import make_identity

    _patch_sem_clear()

    JB = jb_count
    J = JB * 128
    # which j-blocks do lrelu on the Scalar engine (Prelu) vs DVE
    act_lrelu = _knob("GAT_ACT_LRELU", range(JB)[2::3])

    fp32 = mybir.dt.float32
    bf16 = mybir.dt.bfloat16
    A = mybir.AluOpType
    F = mybir.ActivationFunctionType

    nc = bass.Bass(use_seq_codegen=True, detect_race_conditions=RACE_DETECT)

    xk_in = nc.declare_dram_parameter("xk", [J, D], fp32, isOutput=False)
    adjm = nc.declare_dram_parameter("adjm", [J, J], bf16, isOutput=False)
    w_in = nc.declare_dram_parameter("w", [D, D], bf16, isOutput=False)
    wt_in = nc.declare_dram_parameter("wt", [D, D], bf16, isOutput=False)
    alr_in = nc.declare_dram_parameter("alr", [D, 2], bf16, isOutput=False)
    if apply_affine:
        g_in = nc.declare_dram_parameter("gamma", [D], fp32, isOutput=False)
        b_in = nc.declare_dram_parameter("beta", [D], fp32, isOutput=False)
    out_d = nc.declare_dram_parameter("out", [J, D], fp32, isOutput=True)

    el_dram = nc.dram_tensor("el_scratch", [J], bf16)

    # PSUM-bank-aligned i-chunks for matmul outputs
    chunks = []
    s = 0
    while s < J:
        chunks.append((s, min(512, J - s)))
        s += 512

    def bcast(ap, parts=128):
        return bass.AP(tensor=ap.tensor, offset=ap.offset, ap=[[0, parts]] + list(ap.ap))

    with tile.TileContext(nc) as tc:
        with tc.tile_pool(name="persist", bufs=1) as per:
            ident_bf = per.tile([128, 128], bf16)
            make_identity(nc, ident_bf)
            ident_f32 = per.tile([128, 128], fp32)
            make_identity(nc, ident_f32)
            ones_col = per.tile([128, 1], bf16)
            nc.vector.memset(ones_col, 1.0)
            eps_col = per.tile([128, 1], fp32)
            nc.vector.memset(eps_col, EPS)

            w_sb = per.tile([128, D + 2], bf16)   # [W | W@a_l | W@a_r]
            nc.sync.dma_start(out=w_sb[:, :D], in_=w_in[:, :])
            wt_sb = per.tile([128, D], bf16)
            nc.sync.dma_start(out=wt_sb, in_=wt_in[:, :])
            alr_sb = per.tile([128, 2], bf16)
            nc.sync.dma_start(out=alr_sb, in_=alr_in[:, :])
            if apply_affine:
                g_bc = per.tile([128, D], fp32)
                nc.sync.dma_start(out=g_bc, in_=bcast(g_in[:]))
                b_bc = per.tile([128, D], fp32)
                nc.sync.dma_start(out=b_bc, in_=bcast(b_in[:]))

            xk_tiles = [per.tile([128, D], fp32, name=f"xk{i}", tag=f"xk{i}")
                        for i in range(JB)]
            adj_tiles = [per.tile([128, J], bf16, name=f"adj{i}", tag=f"adj{i}")
                         for i in range(JB)]
            xkT_all = per.tile([128, JB, D], bf16)
            h_all = per.tile([128, JB, D], bf16)
            elr_col = per.tile([128, JB, 2], fp32)   # [:, :, 0]=el, [:, :, 1]=er
            erq_col = per.tile([128, JB], fp32)      # 0.2 * er
            el_bc = per.tile([128, J], bf16)
            oT_sb = per.tile([128, J], fp32)
            z_all = per.tile([128, JB, D], fp32)
            o_tiles = [per.tile([128, D], fp32, name=f"o{i}", tag=f"o{i}")
                       for i in range(JB)]
            mv_all = per.tile([128, JB, 2], fp32)
            r_col = per.tile([128, JB], fp32)
            rstd = per.tile([128, JB], fp32)

            # xk first (small, on the critical preproc path), then the big
            # adj tiles behind them on the same queue
            for kb in range(JB):
                nc.sync.dma_start(out=xk_tiles[kb],
                                  in_=xk_in[kb * 128:(kb + 1) * 128, :])
            for jb in range(JB):
                nc.sync.dma_start(out=adj_tiles[jb],
                                  in_=adjm[jb * 128:(jb + 1) * 128, :])

            # ---- preprocessing: xkT, h, el, er --------------------------
            with (
                tc.tile_pool(name="pp", bufs=3) as pp,
                tc.tile_pool(name="pp_ps", bufs=2, space="PSUM") as pp_ps,
                tc.tile_pool(name="pp_ps1", bufs=1, space="PSUM") as pp_ps1,
            ):
                wlr_ps = pp_ps1.tile([128, 2], fp32, tag="wlr")
                nc.tensor.matmul(wlr_ps, lhsT=wt_sb, rhs=alr_sb,
                                 start=True, stop=True)
                nc.vector.tensor_copy(out=w_sb[:, D:D + 2], in_=wlr_ps)

                for kb in range(JB):
                    x_t = xk_tiles[kb]
                    xb = pp.tile([128, D], bf16, tag="xb")
                    nc.gpsimd.tensor_copy(out=xb, in_=x_t)
                    xT_ps = pp_ps.tile([128, D], bf16, tag="xT")
                    nc.tensor.transpose(xT_ps, xb, ident_bf)
                    nc.vector.tensor_copy(out=xkT_all[:, kb, :], in_=xT_ps)
                    he_ps = pp_ps.tile([128, D + 2], fp32, tag="he")
                    nc.tensor.matmul(he_ps, lhsT=xkT_all[:, kb, :], rhs=w_sb,
                                     start=True, stop=True)
                    nc.scalar.copy(out=h_all[:, kb, :], in_=he_ps[:, :D])
                    nc.vector.tensor_copy(out=elr_col[:, kb, :],
                                          in_=he_ps[:, D:D + 2])

                # 0.2*er columns (route-B second tensor_scalar operand)
                nc.vector.tensor_scalar(
                    out=erq_col, in0=elr_col[:, :, 1],
                    scalar1=ALPHA, scalar2=None, op0=A.mult)

                # el column -> broadcast row tile via PE transpose + DRAM bounce
                el_bf = pp.tile([128, JB], bf16, tag="elbf")
                nc.vector.tensor_copy(out=el_bf, in_=elr_col[:, :, 0])
                elT_ps = pp_ps1.tile([JB, 128], bf16, tag="elT")
                nc.tensor.transpose(elT_ps, el_bf, ident_bf)
                elT_sb = pp.tile([JB, 128], bf16, tag="elTs")
                nc.vector.tensor_copy(out=elT_sb, in_=elT_ps)
                nc.gpsimd.dma_start(out=el_dram[:].rearrange("(b q) -> b q", q=128),
                                    in_=elT_sb)
                nc.gpsimd.dma_start(out=el_bc, in_=bcast(el_dram[:]))

            # ---- main loop over j-blocks --------------------------------
            with (
                tc.tile_pool(name="mm_ps", bufs=1, space="PSUM") as mm_ps_pool,
                tc.tile_pool(name="rs_ps", bufs=1, space="PSUM") as rs_ps_pool,
                tc.tile_pool(name="ublk", bufs=4) as ublk,
            ):
                oT_ps = mm_ps_pool.tile([128, J], fp32)
                rs_ps = rs_ps_pool.tile([1, J], fp32)
                for jb in range(JB):
                    adj_t = adj_tiles[jb]
                    er_s = elr_col[:, jb, 1:2]
                    u = ublk.tile([128, J], bf16, tag="u")
                    if jb in act_lrelu:
                        w_t = ublk.tile([128, J], bf16, tag="w")
                        nc.vector.tensor_tensor(out=w_t, in0=adj_t, in1=el_bc,
                                                op=A.add)
                        nc.scalar.activation(out=u, in_=w_t, func=F.Prelu,
                                             bias=er_s, scale=1.0, alpha=ALPHA)
                    else:
                        p = ublk.tile([128, J], bf16, tag="p")
                        nc.vector.tensor_scalar(
                            out=p, in0=el_bc, scalar1=er_s, scalar2=None,
                            op0=A.add)
                        q = ublk.tile([128, J], bf16, tag="q")
                        nc.vector.tensor_scalar(
                            out=q, in0=el_bc, scalar1=ALPHA,
                            scalar2=erq_col[:, jb:jb + 1],
                            op0=A.mult, op1=A.add)
                        u0 = ublk.tile([128, J], bf16, tag="u0")
                        nc.vector.tensor_tensor(out=u0, in0=p, in1=q, op=A.max)
                        nc.vector.tensor_tensor(out=u, in0=u0, in1=adj_t,
                                                op=A.add)
                    pexp = ublk.tile([128, J], bf16, tag="pexp")
                    nc.scalar.activation(out=pexp, in_=u, func=F.Exp)

                    st, sp = (jb == 0), (jb == JB - 1)
                    for cs, cn in chunks:
                        nc.tensor.matmul(oT_ps[:, cs:cs + cn],
                                         lhsT=h_all[:, jb, :],
                                         rhs=pexp[:, cs:cs + cn],
                                         start=st, stop=sp)
                    for cs, cn in chunks:
                        nc.tensor.matmul(rs_ps[:, cs:cs + cn],
                                         lhsT=ones_col,
                                         rhs=pexp[:, cs:cs + cn],
                                         start=st, stop=sp)

                # copy PSUM -> SBUF, split across ACT + DVE (GPSIMD can't
                # read PSUM)
                for idx, (cs, cn) in enumerate(chunks):
                    if idx % 2 == 0:
                        nc.scalar.copy(out=oT_sb[:, cs:cs + cn],
                                       in_=oT_ps[:, cs:cs + cn])
                    else:
                        nc.vector.tensor_copy(out=oT_sb[:, cs:cs + cn],
                                              in_=oT_ps[:, cs:cs + cn])

                # rowsum row [1,J] -> col [128,JB]: PSUM->SBUF dma bounce,
                # PE transpose, reciprocal
                rs_sb = ublk.tile([1, J], fp32, tag="rs_sb")
                nc.scalar.copy(out=rs_sb, in_=rs_ps)
                rsT = ublk.tile([JB, 128], fp32, tag="rsT")
                nc.gpsimd.dma_start(
                    out=rsT, in_=rs_sb[:].rearrange("o (b q) -> o b q", q=128))
                with tc.tile_pool(name="rs2_ps", bufs=1, space="PSUM") as rs2:
                    rsc_ps = rs2.tile([128, JB], fp32, tag="rsc")
                    nc.tensor.transpose(rsc_ps, rsT, ident_f32[:JB, :JB])
                    nc.vector.reciprocal(out=r_col, in_=rsc_ps)

            # ---- epilogue: normalize, residual, layernorm ---------------
            with (
                tc.tile_pool(name="ep", bufs=4) as ep,
                tc.tile_pool(name="ep_ps", bufs=3, space="PSUM") as ep_ps,
            ):
                for ib in range(JB):
                    tr_ps = ep_ps.tile([128, 128], fp32, tag="tr")
                    nc.tensor.transpose(tr_ps, oT_sb[:, ib * 128:(ib + 1) * 128],
                                        ident_f32)
                    z1 = ep.tile([128, 128], fp32, tag="z1")
                    nc.vector.tensor_scalar(out=z1, in0=tr_ps,
                                            scalar1=r_col[:, ib:ib + 1],
                                            scalar2=None, op0=A.mult)
                    nc.vector.tensor_tensor(out=z_all[:, ib, :], in0=z1,
                                            in1=xk_tiles[ib], op=A.add)
                    st6 = ep.tile([128, 6], fp32, tag="st6")
                    nc.vector.bn_stats(out=st6, in_=z_all[:, ib, :])
                    nc.vector.bn_aggr(out=mv_all[:, ib, :], in_=st6)

                # rstd = exp(-0.5*ln(var+eps)) — stays in the ln/exp table
                var_v = mv_all[:, :, 1:2].rearrange("p b o -> p (b o)")
                lnv = ep.tile([128, JB], fp32, tag="lnv")
                nc.scalar.activation(out=lnv, in_=var_v, func=F.Ln,
                                     bias=eps_col, scale=1.0)
                nc.scalar.activation(out=rstd, in_=lnv, func=F.Exp, scale=-0.5)

                for ib in range(JB):
                    o_t = o_tiles[ib]
                    nc.vector.tensor_scalar(
                        out=o_t, in0=z_all[:, ib, :],
                        scalar1=mv_all[:, ib, 0:1].rearrange("p o -> p o"),
                        scalar2=rstd[:, ib:ib + 1],
                        op0=A.subtract, op1=A.mult)
                    if apply_affine:
                        nc.vector.tensor_tensor(out=o_t, in0=o_t, in1=g_bc,
                                                op=A.mult)
                        nc.vector.tensor_tensor(out=o_t, in0=o_t, in1=b_bc,
                                                op=A.add)
                    nc.gpsimd.dma_start(out=out_d[ib * 128:(ib + 1) * 128, :],
                                        in_=o_t)
    return _split_waits(nc, mybir)


def _get_program(jb_count: int, apply_affine: bool):
    key = (jb_count, apply_affine, os.environ.get("GAT_ACT_LRELU"))
    if key not in _PROG_CACHE:
        _PROG_CACHE[key] = _build_program(jb_count, apply_affine)
    return _PROG_CACHE[key]


def _prep_inputs(x, adj_bool, node_mask, W, a_l, a_r, gamma, beta,
                 apply_affine, keeps, J):
    import ml_dtypes

    bf16 = ml_dtypes.bfloat16
    x = np.asarray(x, dtype=np.float32)
    adj_bool = np.asarray(adj_bool)
    w_np = np.asarray(W, dtype=np.float32)
    w_bf = np.ascontiguousarray(w_np.astype(bf16))
    wt_bf = np.ascontiguousarray(w_np.T.astype(bf16))
    alr_bf = np.ascontiguousarray(
        np.stack([np.asarray(a_l, np.float32), np.asarray(a_r, np.float32)],
                 axis=1).astype(bf16))
    in_maps = []
    for b in range(NCORES):
        keep = keeps[b]
        K = len(keep)
        xk = np.zeros((J, D), dtype=np.float32)
        xk[:K] = x[b][keep]
        # adjm[j, i] = 0 if edge(keep_i <- keep_j) else -1e4
        adjm = np.full((J, J), NEG, dtype=np.float32)
        sub = adj_bool[b][np.ix_(keep, keep)]          # [i, j]
        adjm[:K, :K] = (sub.T.astype(np.float32) - 1.0) * (-NEG)
        m = {
            "xk": xk,
            "adjm": np.ascontiguousarray(adjm.astype(bf16)),
            "w": w_bf,
            "wt": wt_bf,
            "alr": alr_bf,
        }
        if apply_affine:
            m["gamma"] = np.ascontiguousarray(np.asarray(gamma, np.float32))
            m["beta"] = np.ascontiguousarray(np.asarray(beta, np.float32))
        in_maps.append(m)
    return in_maps


def kernel(x, adj_bool, node_mask, W, a_l, a_r, gamma, beta):
    global LAST_EXEC_TIME_NS, LAST_MEAN_EXEC_TIME_NS
    from concourse.bass_utils import run_bass_kernel_spmd

    gamma_np = np.asarray(gamma, dtype=np.float32)
    beta_np = np.asarray(beta, dtype=np.float32)
    apply_affine = not (np.all(gamma_np == 1.0) and np.all(beta_np == 0.0))

    node_mask = np.asarray(node_mask)
    keeps = [np.flatnonzero(node_mask[b]) for b in range(NCORES)]
    kmax = max(max(len(k) for k in keeps), 1)
    JB = (kmax + 127) // 128
    J = JB * 128

    nc = _get_program(JB, apply_affine)
    in_maps = _prep_inputs(x, adj_bool, node_mask, W, a_l, a_r,
                           gamma_np, beta_np, apply_affine, keeps, J)
    trace = bool(int(os.environ.get("GAT_TRACE", "0")))
    res = run_bass_kernel_spmd(nc, in_maps, list(range(NCORES)), trace=trace)
    LAST_EXEC_TIME_NS = res.exec_time_ns
    LAST_MEAN_EXEC_TIME_NS = res.mean_exec_time_ns

    out = np.zeros((NCORES, N, D), dtype=np.float32)
    if apply_affine:
        out[:] = beta_np[None, None, :]
    for b in range(NCORES):
        keep = keeps[b]
        dev = np.asarray(res.results[b]["out"], dtype=np.float32)
        out[b][keep] = dev[:len(keep)]
    return out


# revision 26
# speedup vs baseline: 2.4107x; 1.1270x over previous
"""GAT layer (gnn_message_passing) Trainium2 Bass kernel, v2: node-compacted.

Per-core work (data-parallel over batch B=8, one graph per NeuronCore).
Host-side LAYOUT transform: node_mask kills ~50% of nodes; masked rows
and columns of the attention matrix contribute nothing (their pm entries
are zero / their outputs are overwritten by the final mask), so the host
selects the kept node subset and ships compacted tensors:
  xk   [J, D]  kept-node features (zero-padded to J = JB*128)
  adjm [J, J]  additive mask, adjm[j, i] = 0 if edge(keep_i <- keep_j)
               else -1e4, bf16 (j = partition dim, i = free dim)
All model math runs on device, on the compacted graph:
  h    = xk @ W
  e    = lrelu(el_i + er_j + adjm_ji)   (additive mask: lrelu(-1e4+x)
                                         stays hugely negative -> exp=0)
  pm   = exp(e)                         (softmax numerator, pre-masked)
  oT   = h^T @ pm ; rs = 1^T @ pm       (PE, accumulate over j blocks)
  out  = LN(oT^T / rs + xk)
Host scatters kept rows back into the full [N, D] output (masked rows
are exactly beta = LN affine bias, zeros here).

Engine balance: lrelu runs on ScalarE (Prelu, which shares an ACT table
with Exp -> no table reloads) for ACT_LRELU_BLOCKS, on DVE (3-op max
trick) otherwise; small copies are spread over ACT/DVE/Pool.
"""

import os
import sys

import numpy as np

if "/opt/trn_rl_repo" not in sys.path:
    sys.path.insert(0, "/opt/trn_rl_repo")

B, N, D = 8, 2048, 128
ALPHA = 0.2
EPS = 1e-5
NEG = -10000.0
NCORES = 8

_PROG_CACHE = {}
RACE_DETECT = True
SEM_CLEAR_MODE = "skip"  # runtime resets sems between executions (verified)
LAST_EXEC_TIME_NS = None
LAST_MEAN_EXEC_TIME_NS = None


def _knob(name, default):
    v = os.environ.get(name)
    if v is None or v == "":
        return frozenset(default)
    if v == "-":
        return frozenset()
    return frozenset(int(x) for x in v.split(","))


def _patch_sem_clear():
    """This environment's walrus rejects EVENT_SEMAPHORE_RANGE_CLEAR
    ("ISA wrong length").  Tail sem reset is unnecessary here (runtime
    restores sems between executions), so skip it."""
    import bass_rust
    import concourse.bass as bass

    if getattr(bass.BassEngine, "_gat_sem_clear_patched", False):
        return

    def sem_clear(self, sem):
        if SEM_CLEAR_MODE == "skip":
            return None
        if not isinstance(sem, range):
            sem = range(sem.num, sem.num + 1)
        net = {s: 0 for s in sem}
        for b in self.bass.m.functions[0].blocks:
            for inst in b.instructions:
                si = inst.sync_info
                if si is None or not si.on_update:
                    continue
                for u in si.on_update:
                    if u.id in net:
                        if u.update_mode in ("sem-add-imm", "sem-inc"):
                            net[u.id] += u.update_value if u.update_value is not None else 1
                        elif u.update_mode in ("sem-dec",):
                            net[u.id] -= u.update_value if u.update_value is not None else 1
                        else:
                            raise AssertionError(u.update_mode)
        last = None
        for s in sem:
            if net[s]:
                h = bass_rust.SemaphoreHandle(name=f"semdec_{s}", num=s)
                last = self.sem_inc(h, -net[s])
        return last

    bass.BassEngine.sem_clear = sem_clear
    bass.BassEngine._gat_sem_clear_patched = True


def _split_waits(nc, mybir, max_waits=1):
    """This walrus build allows only one semaphore-wait slot per
    instruction; hoist extra waits onto standalone EventSemaphore
    carriers immediately before the offender on the same engine."""
    for f in nc.m.functions:
        for b in f.blocks:
            il = b.instructions
            k = 0
            while k < len(il):
                i = il[k]
                si = i.sync_info
                if si is not None and si.on_wait and len(si.on_wait) > max_waits:
                    waits = list(si.on_wait)
                    extra, keep = waits[:-max_waits], waits[-max_waits:]
                    for j, w in enumerate(extra):
                        ev = mybir.InstEventSemaphore(
                            name=f"{i.name}-wsplit{j}",
                            engine=i.engine,
                            debug=i.debug,
                            sync_info=mybir.SyncInfo(on_wait=[w], on_update=[]),
                        )
                        il.insert(k + j, ev)
                    k += len(extra)
                    i.sync_info = mybir.SyncInfo(
                        on_wait=keep, on_update=list(si.on_update or []))
                k += 1
    return nc


def _build_program(jb_count: int, apply_affine: bool):
    import concourse.bass as bass
    import concourse.tile as tile
    from concourse import mybir
    from concourse.masks import make_identity

    _patch_sem_clear()

    JB = jb_count
    J = JB * 128
    # which j-blocks do lrelu on the Scalar engine (Prelu) vs DVE
    act_lrelu = _knob("GAT_ACT_LRELU", range(JB)[2::3])

    fp32 = mybir.dt.float32
    bf16 = mybir.dt.bfloat16
    A = mybir.AluOpType
    F = mybir.ActivationFunctionType

    nc = bass.Bass(use_seq_codegen=True, detect_race_conditions=RACE_DETECT)

    xk_in = nc.declare_dram_parameter("xk", [J, D], fp32, isOutput=False)
    adjm = nc.declare_dram_parameter("adjm", [J, J], bf16, isOutput=False)
    w_in = nc.declare_dram_parameter("w", [D, D], bf16, isOutput=False)
    wt_in = nc.declare_dram_parameter("wt", [D, D], bf16, isOutput=False)
    alr_in = nc.declare_dram_parameter("alr", [D, 2], bf16, isOutput=False)
    if apply_affine:
        g_in = nc.declare_dram_parameter("gamma", [D], fp32, isOutput=False)
        b_in = nc.declare_dram_parameter("beta", [D], fp32, isOutput=False)
    out_d = nc.declare_dram_parameter("out", [J, D], fp32, isOutput=True)

    el_dram = nc.dram_tensor("el_scratch", [J], bf16)

    # PSUM-bank-aligned i-chunks for matmul outputs
    chunks = []
    s = 0
    while s < J:
        chunks.append((s, min(512, J - s)))
        s += 512

    def bcast(ap, parts=128):
        return bass.AP(tensor=ap.tensor, offset=ap.offset, ap=[[0, parts]] + list(ap.ap))

    with tile.TileContext(nc) as tc:
        with tc.tile_pool(name="persist", bufs=1) as per:
            ident_bf = per.tile([128, 128], bf16)
            make_identity(nc, ident_bf)
            ident_f32 = per.tile([128, 128], fp32)
            make_identity(nc, ident_f32)
            ones_col = per.tile([128, 1], bf16)
            nc.vector.memset(ones_col, 1.0)
            eps_col = per.tile([128, 1], fp32)
            nc.vector.memset(eps_col, EPS)

            w_sb = per.tile([128, D + 2], bf16)   # [W | W@a_l | W@a_r]
            nc.sync.dma_start(out=w_sb[:, :D], in_=w_in[:, :])
            wt_sb = per.tile([128, D], bf16)
            nc.sync.dma_start(out=wt_sb, in_=wt_in[:, :])
            alr_sb = per.tile([128, 2], bf16)
            nc.sync.dma_start(out=alr_sb, in_=alr_in[:, :])
            if apply_affine:
                g_bc = per.tile([128, D], fp32)
                nc.sync.dma_start(out=g_bc, in_=bcast(g_in[:]))
                b_bc = per.tile([128, D], fp32)
                nc.sync.dma_start(out=b_bc, in_=bcast(b_in[:]))

            xk_all = per.tile([128, JB, D], fp32)
            adj_tiles = [per.tile([128, J], bf16, name=f"adj{i}", tag=f"adj{i}")
                         for i in range(JB)]
            xkT_all = per.tile([128, JB, D], bf16)
            h_all = per.tile([128, JB, D], bf16)
            elr_col = per.tile([128, JB, 2], fp32)   # [:, :, 0]=el, [:, :, 1]=er
            erq_col = per.tile([128, JB], fp32)      # 0.2 * er
            el_row = per.tile([1, J], bf16)
            el_bc = per.tile([128, J], bf16)
            oT_sb = per.tile([128, J], fp32)
            z_all = per.tile([128, JB, D], fp32)
            o_all = per.tile([128, JB, D], fp32)
            mv_all = per.tile([128, JB, 2], fp32)
            r_col = per.tile([128, JB], fp32)
            rstd = per.tile([128, JB], fp32)

            # xk in one batched load on the scalar (ACT) HWDGE queue; the big
            # adj tiles stream per-block on the sync queue, starting at t=0
            nc.sync.dma_start(
                out=xk_all, in_=xk_in[:, :].rearrange("(b p) d -> p b d", p=128))
            for jb in range(JB):
                nc.sync.dma_start(out=adj_tiles[jb],
                                  in_=adjm[jb * 128:(jb + 1) * 128, :])

            # ---- preprocessing: xkT, h, el, er --------------------------
            with (
                tc.tile_pool(name="pp", bufs=3) as pp,
                tc.tile_pool(name="pp_ps", bufs=2, space="PSUM") as pp_ps,
                tc.tile_pool(name="pp_ps1", bufs=1, space="PSUM") as pp_ps1,
            ):
                wlr_ps = pp_ps1.tile([128, 2], fp32, tag="wlr")
                nc.tensor.matmul(wlr_ps, lhsT=wt_sb, rhs=alr_sb,
                                 start=True, stop=True)
                nc.vector.tensor_copy(out=w_sb[:, D:D + 2], in_=wlr_ps)

                el_ps = pp_ps1.tile([1, J], fp32, tag="el")
                for kb in range(JB):
                    xb = pp.tile([128, D], bf16, tag="xb")
                    nc.gpsimd.tensor_copy(out=xb, in_=xk_all[:, kb, :])
                    xT_ps = pp_ps.tile([128, D], bf16, tag="xT")
                    nc.tensor.transpose(xT_ps, xb, ident_bf)
                    nc.vector.tensor_copy(out=xkT_all[:, kb, :], in_=xT_ps)
                    he_ps = pp_ps.tile([128, D + 2], fp32, tag="he")
                    nc.tensor.matmul(he_ps, lhsT=xkT_all[:, kb, :], rhs=w_sb,
                                     start=True, stop=True)
                    nc.scalar.copy(out=h_all[:, kb, :], in_=he_ps[:, :D])
                    nc.vector.tensor_copy(out=elr_col[:, kb, :],
                                          in_=he_ps[:, D:D + 2])
                    # el row segment: el[kb*128 : ...] = wl^T @ xkT_kb
                    nc.tensor.matmul(el_ps[:, kb * 128:(kb + 1) * 128],
                                     lhsT=w_sb[:, D:D + 1],
                                     rhs=xkT_all[:, kb, :],
                                     start=True, stop=True)

                # 0.2*er columns (route-B second tensor_scalar operand)
                nc.vector.tensor_scalar(
                    out=erq_col, in0=elr_col[:, :, 1],
                    scalar1=ALPHA, scalar2=None, op0=A.mult)

                # el row -> SBUF bf16 -> DRAM bounce -> partition broadcast
                nc.scalar.copy(out=el_row, in_=el_ps)
                nc.gpsimd.dma_start(
                    out=el_dram[:].rearrange("(o q) -> o q", o=1), in_=el_row)
                nc.gpsimd.dma_start(out=el_bc, in_=bcast(el_dram[:]))

            # ---- main loop over j-blocks --------------------------------
            with (
                tc.tile_pool(name="mm_ps", bufs=1, space="PSUM") as mm_ps_pool,
                tc.tile_pool(name="rs_ps", bufs=1, space="PSUM") as rs_ps_pool,
                tc.tile_pool(name="ublk", bufs=4) as ublk,
            ):
                oT_ps = mm_ps_pool.tile([128, J], fp32)
                rs_ps = rs_ps_pool.tile([1, J], fp32)
                for jb in range(JB):
                    adj_t = adj_tiles[jb]
                    er_s = elr_col[:, jb, 1:2]
                    u = ublk.tile([128, J], bf16, tag="u")
                    if jb in act_lrelu:
                        w_t = ublk.tile([128, J], bf16, tag="w")
                        nc.vector.tensor_tensor(out=w_t, in0=adj_t, in1=el_bc,
                                                op=A.add)
                        nc.scalar.activation(out=u, in_=w_t, func=F.Prelu,
                                             bias=er_s, scale=1.0, alpha=ALPHA)
                    else:
                        p = ublk.tile([128, J], bf16, tag="p")
                        nc.vector.tensor_scalar(
                            out=p, in0=el_bc, scalar1=er_s, scalar2=None,
                            op0=A.add)
                        q = ublk.tile([128, J], bf16, tag="q")
                        nc.vector.tensor_scalar(
                            out=q, in0=el_bc, scalar1=ALPHA,
                            scalar2=erq_col[:, jb:jb + 1],
                            op0=A.mult, op1=A.add)
                        u0 = ublk.tile([128, J], bf16, tag="u0")
                        nc.vector.tensor_tensor(out=u0, in0=p, in1=q, op=A.max)
                        nc.vector.tensor_tensor(out=u, in0=u0, in1=adj_t,
                                                op=A.add)
                    pexp = ublk.tile([128, J], bf16, tag="pexp")
                    nc.scalar.activation(out=pexp, in_=u, func=F.Exp)

                    st, sp = (jb == 0), (jb == JB - 1)
                    # on the last block, finish the rowsums first so the
                    # reciprocal chain overlaps the remaining oT matmuls
                    mm_groups = [
                        (oT_ps, h_all[:, jb, :]), (rs_ps, ones_col)]
                    if sp:
                        mm_groups.reverse()
                    for out_ps, lhs in mm_groups:
                        for cs, cn in chunks:
                            nc.tensor.matmul(out_ps[:, cs:cs + cn],
                                             lhsT=lhs,
                                             rhs=pexp[:, cs:cs + cn],
                                             start=st, stop=sp)

                # copy PSUM -> SBUF, split across ACT + DVE (GPSIMD can't
                # read PSUM)
                for idx, (cs, cn) in enumerate(chunks):
                    if idx % 2 == 0:
                        nc.scalar.copy(out=oT_sb[:, cs:cs + cn],
                                       in_=oT_ps[:, cs:cs + cn])
                    else:
                        nc.vector.tensor_copy(out=oT_sb[:, cs:cs + cn],
                                              in_=oT_ps[:, cs:cs + cn])

                # rowsum row [1,J] -> col [128,JB]: PSUM->SBUF dma bounce,
                # PE transpose, reciprocal
                rs_sb = ublk.tile([1, J], fp32, tag="rs_sb")
                nc.scalar.copy(out=rs_sb, in_=rs_ps)
                rsT = ublk.tile([JB, 128], fp32, tag="rsT")
                nc.gpsimd.dma_start(
                    out=rsT, in_=rs_sb[:].rearrange("o (b q) -> o b q", q=128))
                with tc.tile_pool(name="rs2_ps", bufs=1, space="PSUM") as rs2:
                    rsc_ps = rs2.tile([128, JB], fp32, tag="rsc")
                    nc.tensor.transpose(rsc_ps, rsT, ident_f32[:JB, :JB])
                    nc.vector.reciprocal(out=r_col, in_=rsc_ps)

            # ---- epilogue: normalize, residual, layernorm ---------------
            with (
                tc.tile_pool(name="ep", bufs=4) as ep,
                tc.tile_pool(name="ep_ps", bufs=3, space="PSUM") as ep_ps,
            ):
                for ib in range(JB):
                    tr_ps = ep_ps.tile([128, 128], fp32, tag="tr")
                    nc.tensor.transpose(tr_ps, oT_sb[:, ib * 128:(ib + 1) * 128],
                                        ident_f32)
                    z1 = ep.tile([128, 128], fp32, tag="z1")
                    nc.scalar.activation(out=z1, in_=tr_ps, func=F.Identity,
                                         bias=0.0, scale=r_col[:, ib:ib + 1])
                    nc.gpsimd.tensor_tensor(out=z_all[:, ib, :], in0=z1,
                                            in1=xk_all[:, ib, :], op=A.add)
                    st6 = ep.tile([128, 6], fp32, tag="st6")
                    nc.vector.bn_stats(out=st6, in_=z_all[:, ib, :])
                    nc.vector.bn_aggr(out=mv_all[:, ib, :], in_=st6)

                # rstd = exp(-0.5*ln(var+eps)) — stays in the ln/exp table
                var_v = mv_all[:, :, 1:2].rearrange("p b o -> p (b o)")
                lnv = ep.tile([128, JB], fp32, tag="lnv")
                nc.scalar.activation(out=lnv, in_=var_v, func=F.Ln,
                                     bias=eps_col, scale=1.0)
                nc.scalar.activation(out=rstd, in_=lnv, func=F.Exp, scale=-0.5)

                for ib in range(JB):
                    o_t = o_all[:, ib, :]
                    nc.vector.tensor_scalar(
                        out=o_t, in0=z_all[:, ib, :],
                        scalar1=mv_all[:, ib, 0:1].rearrange("p o -> p o"),
                        scalar2=rstd[:, ib:ib + 1],
                        op0=A.subtract, op1=A.mult)
                    if apply_affine:
                        nc.vector.tensor_tensor(out=o_t, in0=o_t, in1=g_bc,
                                                op=A.mult)
                        nc.vector.tensor_tensor(out=o_t, in0=o_t, in1=b_bc,
                                                op=A.add)
                    if ib % 3 == 2 or ib == JB - 1:
                        lo = (ib // 3) * 3
                        nc.sync.dma_start(
                            out=out_d[lo * 128:(ib + 1) * 128, :].rearrange(
                                "(b p) d -> p b d", p=128),
                            in_=o_all[:, lo:ib + 1, :])
    return _split_waits(nc, mybir)


def _get_program(jb_count: int, apply_affine: bool):
    key = (jb_count, apply_affine, os.environ.get("GAT_ACT_LRELU"))
    if key not in _PROG_CACHE:
        _PROG_CACHE[key] = _build_program(jb_count, apply_affine)
    return _PROG_CACHE[key]


def _prep_inputs(x, adj_bool, node_mask, W, a_l, a_r, gamma, beta,
                 apply_affine, keeps, J):
    import ml_dtypes

    bf16 = ml_dtypes.bfloat16
    x = np.asarray(x, dtype=np.float32)
    adj_bool = np.asarray(adj_bool)
    w_np = np.asarray(W, dtype=np.float32)
    w_bf = np.ascontiguousarray(w_np.astype(bf16))
    wt_bf = np.ascontiguousarray(w_np.T.astype(bf16))
    alr_bf = np.ascontiguousarray(
        np.stack([np.asarray(a_l, np.float32), np.asarray(a_r, np.float32)],
                 axis=1).astype(bf16))
    in_maps = []
    for b in range(NCORES):
        keep = keeps[b]
        K = len(keep)
        xk = np.zeros((J, D), dtype=np.float32)
        xk[:K] = x[b][keep]
        # adjm[j, i] = 0 if edge(keep_i <- keep_j) else -1e4
        adjm = np.full((J, J), NEG, dtype=np.float32)
        sub = adj_bool[b][np.ix_(keep, keep)]          # [i, j]
        adjm[:K, :K] = (sub.T.astype(np.float32) - 1.0) * (-NEG)
        m = {
            "xk": xk,
            "adjm": np.ascontiguousarray(adjm.astype(bf16)),
            "w": w_bf,
            "wt": wt_bf,
            "alr": alr_bf,
        }
        if apply_affine:
            m["gamma"] = np.ascontiguousarray(np.asarray(gamma, np.float32))
            m["beta"] = np.ascontiguousarray(np.asarray(beta, np.float32))
        in_maps.append(m)
    return in_maps


def kernel(x, adj_bool, node_mask, W, a_l, a_r, gamma, beta):
    global LAST_EXEC_TIME_NS, LAST_MEAN_EXEC_TIME_NS
    from concourse.bass_utils import run_bass_kernel_spmd

    gamma_np = np.asarray(gamma, dtype=np.float32)
    beta_np = np.asarray(beta, dtype=np.float32)
    apply_affine = not (np.all(gamma_np == 1.0) and np.all(beta_np == 0.0))

    node_mask = np.asarray(node_mask)
    keeps = [np.flatnonzero(node_mask[b]) for b in range(NCORES)]
    kmax = max(max(len(k) for k in keeps), 1)
    JB = (kmax + 127) // 128
    J = JB * 128

    nc = _get_program(JB, apply_affine)
    in_maps = _prep_inputs(x, adj_bool, node_mask, W, a_l, a_r,
                           gamma_np, beta_np, apply_affine, keeps, J)
    trace = bool(int(os.environ.get("GAT_TRACE", "0")))
    res = run_bass_kernel_spmd(nc, in_maps, list(range(NCORES)), trace=trace)
    LAST_EXEC_TIME_NS = res.exec_time_ns
    LAST_MEAN_EXEC_TIME_NS = res.mean_exec_time_ns

    out = np.zeros((NCORES, N, D), dtype=np.float32)
    if apply_affine:
        out[:] = beta_np[None, None, :]
    for b in range(NCORES):
        keep = keeps[b]
        dev = np.asarray(res.results[b]["out"], dtype=np.float32)
        out[b][keep] = dev[:len(keep)]
    return out
